# revision 13
# baseline (speedup 1.0000x reference)
"""Trainium2 Bass kernel for nn_AttentionSimple (sparse_attention, 8 cores).

Reference (per batch row b):
    e      = embeddings[k[b]]              # [S, E] gather
    scores = q[b] . e[s]                   # [S]
    attn   = softmax(scores); ctx = sum_s attn[s] * e[s]
    out    = ctx @ W.T + b                 # [B, 2]

Algorithm: count-weighted vocab-space softmax — no per-token gathers.
Scores depend on s only through v = k[b, s], so group softmax terms by
vocabulary id:
    c[b, v]  = |{s : k[b, s] = v}|         (histogram of k, built on host
                                            during input sharding)
    l[b, v]  = q[b] . embeddings[v]        (dense PE matmul)
    A        = c * exp(l)
    out[b]   = (sum_v A[b,v] * EW[v]) / (sum_v A[b,v])
    with EW  = embeddings @ W.T + b        (parameter prepacking, host)

Sharding: padded vocabulary (53248 = 416 chunks of 128) split across 8
cores (52 chunks each); every core handles all 128 batch rows. Cores
return partial numerators/denominators; host sums and divides.

Per-core pipeline (all operands bf16 unless noted):
  - embT [100, 3328]: chunk PAIRS stacked on the contraction dim (even
    chunk at partitions 0:50, odd at 50:100 — no padding bytes).
  - mm1: ps[128, 256] = etpair.T @ qw (qw = block-diag [qT|qT]); 4
    pairs fill a [128, 1024] PSUM tile (an "oct" = 8 chunks, 2 banks).
  - counts: octs 0-2 multiply u8 counts on DVE after exp; octs 3-6
    fold ln(c) (fp8, -64 for c=0) into the PSUM via identity matmuls
    BEFORE exp (exp(l + ln c) = c exp(l)), region-matched 256 wide to
    each mm1 (PSUM accumulates must exactly match the region their
    start=True matmul opened, and PSUM bank 0 corrupts accumulation
    chains entirely — it is parked for warmup/gate use only).
  - ACT: le = exp(ps), 1024 wide, bf16 out.
  - mm2: acc[9, 1024] += st9_half.T @ le_half (512 cols); st9 =
    [EW c0..c3 | ones]; even halves to acc[:, 0:512], odd to 512:1024.
  - DMA: descriptor feed is ~12ns/descriptor per queue, so effective
    bandwidth = row_bytes/12ns — few BIG transfers win. qw rides fused
    with oct0's embeddings (fat rows) first on the sync queue, then
    two et bulk chunks; u8 counts and fp8 ln-counts are single whole
    transfers on the gpsimd queue; identity/st9 on the scalar queue.
  - gate matmul just before the first folded oct consumes the id/lnc
    DMA semaphores on the Tensor queue at a region-isolated spot
    (parked bank0) so fold matmuls never carry sem waits (the PE
    wait-queue can bypass-reorder waiting instructions, which would
    corrupt the region-matched accumulation chains).
  - PE warmup: matmuls on a memset tile so the PE p-state ramp
    (0.65 -> 2.4 GHz over ~3us of busy time) overlaps the input DMAs.
"""

import numpy as np

BATCH, SEQ, EMB, VOCAB, OUT = 128, 8192, 50, 50000, 2
N_CORES = 8
CSH = 52                         # vocab chunks per core
NCHUNK = CSH * N_CORES           # 416
VPAD = NCHUNK * 128              # 53248
VSH = CSH * 128                  # 6656
NPAIR = CSH // 2                 # 26 pair-columns of 128 ids
NHALF = CSH // 4                 # 13 half-octs (4 chunks = 512 le cols)
NQW = 2 * BATCH                  # 256 moving columns of mm1
ETP = 2 * EMB                    # 100 real contraction partitions

# octs: groups of 4 pairs (8 chunks); last oct has 2 pairs
OCT_PAIRS = [4, 4, 4, 4, 4, 4, 2]
NOCT = len(OCT_PAIRS)
FOLD_OCT0 = 3                    # first oct using the ln(c) PE fold
CT_COLS = FOLD_OCT0 * 1024       # 3072 u8 count columns
N_WARMUP = 6

_CACHE = {}


def _build_nc():
    from contextlib import ExitStack

    import concourse.mybir as mybir
    import concourse.tile as tile
    from concourse import bacc

    f32 = mybir.dt.float32
    bf16 = mybir.dt.bfloat16
    u8 = mybir.dt.uint8
    f8 = mybir.dt.float8e4
    nc = bacc.Bacc("TRN2", target_bir_lowering=False, debug=False,
                   num_devices=N_CORES)

    # qw fused with oct0's embedding pairs: fat rows, lands first
    qe_d = nc.dram_tensor("qe", [ETP, NQW + 512], bf16,
                          kind="ExternalInput")
    embT_d = nc.dram_tensor("embT", [ETP, NPAIR * 128], bf16,
                            kind="ExternalInput")
    # combo plane: [u8 counts | st9 bf16 bytes | identity fp8 bytes]
    CT_XTRA = NHALF * 9 * 2 + 128
    ct_d = nc.dram_tensor("ct", [128, CT_COLS + CT_XTRA], u8,
                          kind="ExternalInput")
    lnc_d = nc.dram_tensor("lnc", [128, VSH - CT_COLS], f8,
                           kind="ExternalInput")
    o_d = nc.dram_tensor("o", [9, 1024], f32, kind="ExternalOutput")

    oct_pair0 = np.cumsum([0] + OCT_PAIRS).tolist()

    with tile.TileContext(nc) as tc, ExitStack() as ctx:
        const_p = ctx.enter_context(tc.tile_pool(name="const", bufs=1))
        et_p = ctx.enter_context(tc.tile_pool(name="etp", bufs=2))
        ct_p = ctx.enter_context(tc.tile_pool(name="ctp", bufs=1))
        le_p = ctx.enter_context(tc.tile_pool(name="le", bufs=4))
        park_p = ctx.enter_context(tc.tile_pool(name="park", bufs=1,
                                                space="PSUM"))
        ps_p = ctx.enter_context(tc.tile_pool(name="ps", bufs=2,
                                              space="PSUM"))
        acc_p = ctx.enter_context(tc.tile_pool(name="acc", bufs=1,
                                               space="PSUM"))
        fin_p = ctx.enter_context(tc.tile_pool(name="fin", bufs=1))

        # PE warmup on a memset tile in the parked bank0
        wtile = const_p.tile([128, 512], bf16)
        nc.gpsimd.memset(wtile[:], 0.0)
        wps = park_p.tile([128, 512], f32)
        for _ in range(N_WARMUP):
            nc.tensor.matmul(wps[:], lhsT=wtile[:, 0:128],
                             rhs=wtile[:], start=True, stop=True)

        # sync queue: fused qw+oct0, then the first et bulk chunk;
        # scalar queue: the second et bulk (parallel queues get separate
        # engine shares, so et is not starved by the count planes)
        qe_sb = const_p.tile([ETP, NQW + 512], bf16)
        nc.sync.dma_start(qe_sb[:], qe_d.ap())
        qw_sb = qe_sb[:, 0:NQW]
        et_tiles = []
        for eng, (o0, o1) in ((nc.sync, (1, 3)), (nc.scalar, (3, NOCT))):
            p0, p1 = oct_pair0[o0], oct_pair0[o1]
            t = et_p.tile([ETP, 14 * 128], bf16, tag="et")
            eng.dma_start(t[:, 0:(p1 - p0) * 128],
                          embT_d.ap()[:, p0 * 128:p1 * 128])
            et_tiles.append((p0, p1, t))

        # gpsimd queue: combo plane (counts + st9 + identity share one
        # fat-row transfer), then fp8 ln-counts in two chunks (each fold
        # oct only gates on the chunk it needs)
        ct_sb = ct_p.tile([128, CT_COLS + CT_XTRA], u8)
        nc.gpsimd.dma_start(ct_sb[:], ct_d.ap())
        st_sb = ct_sb[:, CT_COLS:CT_COLS + NHALF * 9 * 2].bitcast(bf16)
        id_sb = ct_sb[:, CT_COLS + NHALF * 9 * 2:].bitcast(f8)
        lnc_sb = ct_p.tile([128, VSH - CT_COLS], f8)
        LNC_SPLIT = 1024         # oct3's plane first, rest after
        nc.gpsimd.dma_start(lnc_sb[:, 0:LNC_SPLIT],
                            lnc_d.ap()[:, 0:LNC_SPLIT])
        nc.gpsimd.dma_start(lnc_sb[:, LNC_SPLIT:],
                            lnc_d.ap()[:, LNC_SPLIT:])

        def et_slice(pair):
            if pair < 4:
                return qe_sb[:, NQW + pair * 128:NQW + (pair + 1) * 128]
            for (p0, p1, t) in et_tiles:
                if p0 <= pair < p1:
                    c = (pair - p0) * 128
                    return t[:, c:c + 128]
            raise AssertionError(pair)

        def lnc_slice(o, lp):
            c0 = oct_pair0[o] * 256 - CT_COLS + lp * 256
            return lnc_sb[:, c0:c0 + 256]

        acc = acc_p.tile([9, 1024], f32)
        le_tiles = [None] * NOCT
        acc_started = [False, False]
        last_half = {0: max(h for h in range(NHALF) if h % 2 == 0),
                     1: max(h for h in range(NHALF) if h % 2 == 1)}

        def emit_mm2(o):
            le = le_tiles[o]
            npair = OCT_PAIRS[o]
            for hh in range(npair // 2):
                half = oct_pair0[o] // 2 + hh
                bank = half % 2
                nc.tensor.matmul(
                    acc[:, bank * 512:(bank + 1) * 512],
                    lhsT=st_sb[:, half * 9:(half + 1) * 9],
                    rhs=le[:, hh * 512:(hh + 1) * 512],
                    start=not acc_started[bank],
                    stop=half == last_half[bank],
                    skip_group_check=True,
                )
                acc_started[bank] = True

        for o in range(NOCT):
            npair = OCT_PAIRS[o]
            folded = o >= FOLD_OCT0
            if o == FOLD_OCT0:
                # gates: consume the id+lnc DMA semaphores on the Tensor
                # queue in the parked bank before any fold needs them
                nc.tensor.matmul(wps[:, 0:256], lhsT=id_sb,
                                 rhs=lnc_sb[:, 0:256],
                                 start=True, stop=True,
                                 skip_group_check=True)
            if o == FOLD_OCT0 + 1:
                nc.tensor.matmul(wps[:, 0:256], lhsT=id_sb,
                                 rhs=lnc_sb[:, 1024:1280],
                                 start=True, stop=True,
                                 skip_group_check=True)
            ps = ps_p.tile([128, 1024], f32, tag="ps")
            for lp in range(npair):
                pair = oct_pair0[o] + lp
                nc.tensor.matmul(
                    ps[:, lp * 256:(lp + 1) * 256],
                    lhsT=et_slice(pair),
                    rhs=qw_sb,
                    start=True, stop=not folded,
                    skip_group_check=True,
                )
                if folded:
                    # += ln(c) in the SAME 256-col region (accumulates
                    # must region-match their start=True matmul)
                    nc.tensor.matmul(
                        ps[:, lp * 256:(lp + 1) * 256],
                        lhsT=id_sb,
                        rhs=lnc_slice(o, lp),
                        start=False, stop=True,
                        skip_group_check=True,
                    )
            le = le_p.tile([128, 1024], bf16, tag="le")
            le_tiles[o] = le
            w = npair * 256
            nc.scalar.activation(le[:, 0:w], ps[:, 0:w],
                                 mybir.ActivationFunctionType.Exp)
            if not folded:
                for hh in range(npair // 2):
                    half = oct_pair0[o] // 2 + hh
                    nc.vector.tensor_mul(
                        le[:, hh * 512:(hh + 1) * 512],
                        le[:, hh * 512:(hh + 1) * 512],
                        ct_sb[:, half * 512:(half + 1) * 512])
            if o >= 3:
                emit_mm2(o - 3)
        emit_mm2(NOCT - 3)
        emit_mm2(NOCT - 2)
        emit_mm2(NOCT - 1)

        osb = fin_p.tile([9, 1024], f32)
        nc.scalar.activation(osb[:], acc[:],
                             mybir.ActivationFunctionType.Copy)
        nc.sync.dma_start(o_d.ap(), osb[:])

    nc.finalize()
    return nc


def _prep_inputs(q, k, embeddings, W, b):
    import ml_dtypes

    q = np.ascontiguousarray(q, dtype=np.float32)
    emb = np.ascontiguousarray(embeddings, dtype=np.float32)
    W = np.ascontiguousarray(W, dtype=np.float32)
    b = np.ascontiguousarray(b, dtype=np.float32)
    k = np.asarray(k)

    embT = np.zeros((EMB, VPAD), np.float32)
    embT[:, :VOCAB] = emb.T

    # mm1 moving operand: block-diagonal [qT | 0; 0 | qT]
    qw = np.zeros((ETP, NQW), np.float32)
    qw[:EMB, 0:BATCH] = q.T
    qw[EMB:ETP, BATCH:2 * BATCH] = q.T

    EWp = np.zeros((VPAD, OUT), np.float32)
    EWp[:VOCAB] = emb @ W.T + b[None, :]

    flat = (np.arange(BATCH, dtype=np.int64)[:, None] * VPAD
            + k.astype(np.int64)).ravel()
    C = np.bincount(flat, minlength=BATCH * VPAD).reshape(BATCH, VPAD)
    assert C.max() <= 255, "count histogram overflows uint8 transport"

    in_maps = []
    for core in range(N_CORES):
        v0 = core * VSH
        blocks = embT[:, v0:v0 + VSH].reshape(EMB, CSH, 128)
        e2 = np.zeros((ETP, NPAIR, 128), np.float32)
        e2[:EMB] = blocks[:, 0::2, :]
        e2[EMB:ETP] = blocks[:, 1::2, :]
        e2 = e2.reshape(ETP, NPAIR * 128)
        qe = np.ascontiguousarray(
            np.concatenate([qw, e2[:, 0:512]], axis=1)
        ).astype(ml_dtypes.bfloat16)
        e2 = np.ascontiguousarray(e2).astype(ml_dtypes.bfloat16)

        # st9 per half-oct: cols 2j+o = EW[chunk 4h+j, o]; col 8 = 1
        ew_blocks = EWp[v0:v0 + VSH].reshape(CSH, 128, OUT)
        st = np.zeros((128, NHALF, 9), np.float32)
        for j in range(4):
            st[:, :, 2 * j:2 * j + 2] = (
                ew_blocks.reshape(NHALF, 4, 128, OUT)[:, j]
                .transpose(1, 0, 2))
        st[:, :, 8] = 1.0
        st = np.ascontiguousarray(
            st.reshape(128, NHALF * 9)).astype(ml_dtypes.bfloat16)

        Cc = (C[:, v0:v0 + VSH].reshape(BATCH, CSH, 128)
              .transpose(2, 1, 0).reshape(128, CSH * BATCH))
        Cf = Cc[:, CT_COLS:].astype(np.float64)
        lnc = np.where(Cf > 0, np.log(np.maximum(Cf, 1e-30)), -64.0)
        lnc = np.ascontiguousarray(lnc.astype(ml_dtypes.float8_e4m3))
        ident = np.eye(128, dtype=ml_dtypes.float8_e4m3)
        ct = np.ascontiguousarray(np.concatenate(
            [Cc[:, :CT_COLS].astype(np.uint8),
             st.view(np.uint8), ident.view(np.uint8)], axis=1))
        in_maps.append({"qe": qe, "embT": e2, "ct": ct, "lnc": lnc})
    return in_maps


def _run_device(in_maps, **kwargs):
    from concourse.bass_utils import run_bass_kernel_spmd

    if "nc" not in _CACHE:
        _CACHE["nc"] = _build_nc()
    return run_bass_kernel_spmd(_CACHE["nc"], in_maps,
                                core_ids=list(range(N_CORES)), **kwargs)


def _unshard(res):
    P = np.zeros((9, 1024), np.float64)
    for i in range(N_CORES):
        P += res.results[i]["o"].astype(np.float64)
    numer = np.zeros((OUT, BATCH), np.float64)
    denom = np.zeros(BATCH, np.float64)
    for bank in range(2):
        Pb = P[:, bank * 512:(bank + 1) * 512]
        for j in range(4):
            numer += Pb[2 * j:2 * j + 2, j * BATCH:(j + 1) * BATCH]
            denom += Pb[8, j * BATCH:(j + 1) * BATCH]
    out = (numer / denom[None, :]).T
    return np.ascontiguousarray(out, dtype=np.float32)


def kernel(q, k, embeddings, W, b, **_unused):
    in_maps = _prep_inputs(q, k, embeddings, W, b)
    res = _run_device(in_maps)
    return _unshard(res)


# revision 14
# speedup vs baseline: 1.1225x; 1.1225x over previous
"""Trainium2 Bass kernel for nn_AttentionSimple (sparse_attention, 8 cores).

Reference (per batch row b):
    e      = embeddings[k[b]]              # [S, E] gather
    scores = q[b] . e[s]                   # [S]
    attn   = softmax(scores); ctx = sum_s attn[s] * e[s]
    out    = ctx @ W.T + b                 # [B, 2]

Algorithm: count-weighted vocab-space softmax — no per-token gathers.
Scores depend on s only through v = k[b, s], so group softmax terms by
vocabulary id:
    c[b, v]  = |{s : k[b, s] = v}|         (histogram of k, built on host
                                            during input sharding)
    l[b, v]  = q[b] . embeddings[v]        (dense PE matmul)
    A        = c * exp(l)
    out[b]   = (sum_v A[b,v] * EW[v]) / (sum_v A[b,v])
    with EW  = embeddings @ W.T + b        (parameter prepacking, host)

Sharding: padded vocabulary (53248 = 416 chunks of 128) split across 8
cores (52 chunks each); every core handles all 128 batch rows. Cores
return partial numerators/denominators; host sums and divides.

Per-core pipeline (all operands bf16 unless noted):
  - embT [100, 3328]: chunk PAIRS stacked on the contraction dim (even
    chunk at partitions 0:50, odd at 50:100 — no padding bytes).
  - mm1: ps[128, 256] = etpair.T @ qw (qw = block-diag [qT|qT]); 4
    pairs fill a [128, 1024] PSUM tile (an "oct" = 8 chunks, 2 banks).
  - counts: octs 0-2 multiply u8 counts on DVE after exp; octs 3-6
    fold ln(c) (fp8, -64 for c=0) into the PSUM via identity matmuls
    BEFORE exp (exp(l + ln c) = c exp(l)), region-matched 256 wide to
    each mm1 (PSUM accumulates must exactly match the region their
    start=True matmul opened, and PSUM bank 0 corrupts accumulation
    chains entirely — it is parked for warmup/gate use only).
  - ACT: le = exp(ps), 1024 wide, bf16 out.
  - mm2: acc[9, 1024] += st9_half.T @ le_half (512 cols); st9 =
    [EW c0..c3 | ones]; even halves to acc[:, 0:512], odd to 512:1024.
  - DMA: descriptor feed is ~12ns/descriptor per queue, so effective
    bandwidth = row_bytes/12ns — few BIG transfers win. qw rides fused
    with oct0's embeddings (fat rows) first on the sync queue, then
    two et bulk chunks; u8 counts and fp8 ln-counts are single whole
    transfers on the gpsimd queue; identity/st9 on the scalar queue.
  - gate matmul just before the first folded oct consumes the id/lnc
    DMA semaphores on the Tensor queue at a region-isolated spot
    (parked bank0) so fold matmuls never carry sem waits (the PE
    wait-queue can bypass-reorder waiting instructions, which would
    corrupt the region-matched accumulation chains).
  - PE warmup: matmuls on a memset tile so the PE p-state ramp
    (0.65 -> 2.4 GHz over ~3us of busy time) overlaps the input DMAs.
"""

import numpy as np

BATCH, SEQ, EMB, VOCAB, OUT = 128, 8192, 50, 50000, 2
N_CORES = 8
CSH = 52                         # vocab chunks per core
NCHUNK = CSH * N_CORES           # 416
VPAD = NCHUNK * 128              # 53248
VSH = CSH * 128                  # 6656
NPAIR = CSH // 2                 # 26 pair-columns of 128 ids
NHALF = CSH // 4                 # 13 half-octs (4 chunks = 512 le cols)
NQW = 2 * BATCH                  # 256 moving columns of mm1
ETP = 2 * EMB                    # 100 real contraction partitions

# octs: groups of 4 pairs (8 chunks); last oct has 2 pairs
OCT_PAIRS = [4, 4, 4, 4, 4, 4, 2]
NOCT = len(OCT_PAIRS)
FOLD_OCT0 = 3                    # first oct using the ln(c) PE fold
CT_COLS = FOLD_OCT0 * 1024       # 3072 u8 count columns
N_WARMUP = 6

_CACHE = {}


def _build_nc():
    from contextlib import ExitStack

    import concourse.mybir as mybir
    import concourse.tile as tile
    from concourse import bacc

    f32 = mybir.dt.float32
    bf16 = mybir.dt.bfloat16
    u8 = mybir.dt.uint8
    f8 = mybir.dt.float8e4
    nc = bacc.Bacc("TRN2", target_bir_lowering=False, debug=False,
                   num_devices=N_CORES)

    # qw fused with oct0's embedding pairs: fat rows, lands first
    qe_d = nc.dram_tensor("qe", [ETP, NQW + 512], bf16,
                          kind="ExternalInput")
    embT_d = nc.dram_tensor("embT", [ETP, NPAIR * 128], bf16,
                            kind="ExternalInput")
    # combo plane: [u8 counts | st9 bf16 bytes | identity fp8 bytes]
    CT_XTRA = NHALF * 9 * 2 + 128
    ct_d = nc.dram_tensor("ct", [128, CT_COLS + CT_XTRA], u8,
                          kind="ExternalInput")
    lnc_d = nc.dram_tensor("lnc", [128, VSH - CT_COLS], f8,
                           kind="ExternalInput")
    o_d = nc.dram_tensor("o", [9, 1024], f32, kind="ExternalOutput")

    oct_pair0 = np.cumsum([0] + OCT_PAIRS).tolist()

    with tile.TileContext(nc) as tc, ExitStack() as ctx:
        const_p = ctx.enter_context(tc.tile_pool(name="const", bufs=1))
        et_p = ctx.enter_context(tc.tile_pool(name="etp", bufs=4))
        ct_p = ctx.enter_context(tc.tile_pool(name="ctp", bufs=1))
        le_p = ctx.enter_context(tc.tile_pool(name="le", bufs=4))
        park_p = ctx.enter_context(tc.tile_pool(name="park", bufs=1,
                                                space="PSUM"))
        ps_p = ctx.enter_context(tc.tile_pool(name="ps", bufs=2,
                                              space="PSUM"))
        acc_p = ctx.enter_context(tc.tile_pool(name="acc", bufs=1,
                                               space="PSUM"))
        fin_p = ctx.enter_context(tc.tile_pool(name="fin", bufs=1))

        # PE warmup on a memset tile in the parked bank0
        wtile = const_p.tile([128, 512], bf16)
        nc.gpsimd.memset(wtile[:], 0.0)
        wps = park_p.tile([128, 512], f32)
        for _ in range(N_WARMUP):
            nc.tensor.matmul(wps[:], lhsT=wtile[:, 0:128],
                             rhs=wtile[:], start=True, stop=True)

        # sync queue: fused qw+oct0, then the first et bulk chunk;
        # scalar queue: the second et bulk (parallel queues get separate
        # engine shares, so et is not starved by the count planes)
        qe_sb = const_p.tile([ETP, NQW + 512], bf16)
        nc.sync.dma_start(qe_sb[:], qe_d.ap())
        qw_sb = qe_sb[:, 0:NQW]
        et_tiles = []
        for (o0, o1) in ((1, 2), (2, 3), (3, 5), (5, NOCT)):
            p0, p1 = oct_pair0[o0], oct_pair0[o1]
            t = et_p.tile([ETP, 8 * 128], bf16, tag="et")
            nc.sync.dma_start(t[:, 0:(p1 - p0) * 128],
                              embT_d.ap()[:, p0 * 128:p1 * 128])
            et_tiles.append((p0, p1, t))

        # gpsimd queue: combo plane (counts + st9 + identity share one
        # fat-row transfer), then fp8 ln-counts in two chunks (each fold
        # oct only gates on the chunk it needs)
        ct_sb = ct_p.tile([128, CT_COLS + CT_XTRA], u8)
        nc.gpsimd.dma_start(ct_sb[:], ct_d.ap())
        st_sb = ct_sb[:, CT_COLS:CT_COLS + NHALF * 9 * 2].bitcast(bf16)
        id_sb = ct_sb[:, CT_COLS + NHALF * 9 * 2:].bitcast(f8)
        lnc_sb = ct_p.tile([128, VSH - CT_COLS], f8)
        LNC_SPLIT = 1024         # oct3's plane first, rest after
        nc.gpsimd.dma_start(lnc_sb[:, 0:LNC_SPLIT],
                            lnc_d.ap()[:, 0:LNC_SPLIT])
        nc.gpsimd.dma_start(lnc_sb[:, LNC_SPLIT:],
                            lnc_d.ap()[:, LNC_SPLIT:])

        def et_slice(pair):
            if pair < 4:
                return qe_sb[:, NQW + pair * 128:NQW + (pair + 1) * 128]
            for (p0, p1, t) in et_tiles:
                if p0 <= pair < p1:
                    c = (pair - p0) * 128
                    return t[:, c:c + 128]
            raise AssertionError(pair)

        def lnc_slice(o, lp):
            c0 = oct_pair0[o] * 256 - CT_COLS + lp * 256
            return lnc_sb[:, c0:c0 + 256]

        acc = acc_p.tile([9, 1024], f32)
        le_tiles = [None] * NOCT
        acc_started = [False, False]
        last_half = {0: max(h for h in range(NHALF) if h % 2 == 0),
                     1: max(h for h in range(NHALF) if h % 2 == 1)}

        def emit_mm2(o):
            le = le_tiles[o]
            npair = OCT_PAIRS[o]
            for hh in range(npair // 2):
                half = oct_pair0[o] // 2 + hh
                bank = half % 2
                nc.tensor.matmul(
                    acc[:, bank * 512:(bank + 1) * 512],
                    lhsT=st_sb[:, half * 9:(half + 1) * 9],
                    rhs=le[:, hh * 512:(hh + 1) * 512],
                    start=not acc_started[bank],
                    stop=half == last_half[bank],
                    skip_group_check=True,
                )
                acc_started[bank] = True

        for o in range(NOCT):
            npair = OCT_PAIRS[o]
            folded = o >= FOLD_OCT0
            if o == FOLD_OCT0:
                # gates: consume the id+lnc DMA semaphores on the Tensor
                # queue in the parked bank before any fold needs them
                nc.tensor.matmul(wps[:, 0:256], lhsT=id_sb,
                                 rhs=lnc_sb[:, 0:256],
                                 start=True, stop=True,
                                 skip_group_check=True)
            if o == FOLD_OCT0 + 1:
                nc.tensor.matmul(wps[:, 0:256], lhsT=id_sb,
                                 rhs=lnc_sb[:, 1024:1280],
                                 start=True, stop=True,
                                 skip_group_check=True)
            ps = ps_p.tile([128, 1024], f32, tag="ps")
            for lp in range(npair):
                pair = oct_pair0[o] + lp
                nc.tensor.matmul(
                    ps[:, lp * 256:(lp + 1) * 256],
                    lhsT=et_slice(pair),
                    rhs=qw_sb,
                    start=True, stop=not folded,
                    skip_group_check=True,
                )
                if folded:
                    # += ln(c) in the SAME 256-col region (accumulates
                    # must region-match their start=True matmul)
                    nc.tensor.matmul(
                        ps[:, lp * 256:(lp + 1) * 256],
                        lhsT=id_sb,
                        rhs=lnc_slice(o, lp),
                        start=False, stop=True,
                        skip_group_check=True,
                    )
            le = le_p.tile([128, 1024], bf16, tag="le")
            le_tiles[o] = le
            w = npair * 256
            nc.scalar.activation(le[:, 0:w], ps[:, 0:w],
                                 mybir.ActivationFunctionType.Exp)
            if not folded:
                for hh in range(npair // 2):
                    half = oct_pair0[o] // 2 + hh
                    nc.vector.tensor_mul(
                        le[:, hh * 512:(hh + 1) * 512],
                        le[:, hh * 512:(hh + 1) * 512],
                        ct_sb[:, half * 512:(half + 1) * 512])
            if o >= 3:
                emit_mm2(o - 3)
        emit_mm2(NOCT - 3)
        emit_mm2(NOCT - 2)
        emit_mm2(NOCT - 1)

        osb = fin_p.tile([9, 1024], f32)
        nc.scalar.activation(osb[:], acc[:],
                             mybir.ActivationFunctionType.Copy)
        nc.sync.dma_start(o_d.ap(), osb[:])

    nc.finalize()
    return nc


def _prep_inputs(q, k, embeddings, W, b):
    import ml_dtypes

    q = np.ascontiguousarray(q, dtype=np.float32)
    emb = np.ascontiguousarray(embeddings, dtype=np.float32)
    W = np.ascontiguousarray(W, dtype=np.float32)
    b = np.ascontiguousarray(b, dtype=np.float32)
    k = np.asarray(k)

    embT = np.zeros((EMB, VPAD), np.float32)
    embT[:, :VOCAB] = emb.T

    # mm1 moving operand: block-diagonal [qT | 0; 0 | qT]
    qw = np.zeros((ETP, NQW), np.float32)
    qw[:EMB, 0:BATCH] = q.T
    qw[EMB:ETP, BATCH:2 * BATCH] = q.T

    EWp = np.zeros((VPAD, OUT), np.float32)
    EWp[:VOCAB] = emb @ W.T + b[None, :]

    flat = (np.arange(BATCH, dtype=np.int64)[:, None] * VPAD
            + k.astype(np.int64)).ravel()
    C = np.bincount(flat, minlength=BATCH * VPAD).reshape(BATCH, VPAD)
    assert C.max() <= 255, "count histogram overflows uint8 transport"

    in_maps = []
    for core in range(N_CORES):
        v0 = core * VSH
        blocks = embT[:, v0:v0 + VSH].reshape(EMB, CSH, 128)
        e2 = np.zeros((ETP, NPAIR, 128), np.float32)
        e2[:EMB] = blocks[:, 0::2, :]
        e2[EMB:ETP] = blocks[:, 1::2, :]
        e2 = e2.reshape(ETP, NPAIR * 128)
        qe = np.ascontiguousarray(
            np.concatenate([qw, e2[:, 0:512]], axis=1)
        ).astype(ml_dtypes.bfloat16)
        e2 = np.ascontiguousarray(e2).astype(ml_dtypes.bfloat16)

        # st9 per half-oct: cols 2j+o = EW[chunk 4h+j, o]; col 8 = 1
        ew_blocks = EWp[v0:v0 + VSH].reshape(CSH, 128, OUT)
        st = np.zeros((128, NHALF, 9), np.float32)
        for j in range(4):
            st[:, :, 2 * j:2 * j + 2] = (
                ew_blocks.reshape(NHALF, 4, 128, OUT)[:, j]
                .transpose(1, 0, 2))
        st[:, :, 8] = 1.0
        st = np.ascontiguousarray(
            st.reshape(128, NHALF * 9)).astype(ml_dtypes.bfloat16)

        Cc = (C[:, v0:v0 + VSH].reshape(BATCH, CSH, 128)
              .transpose(2, 1, 0).reshape(128, CSH * BATCH))
        Cf = Cc[:, CT_COLS:].astype(np.float64)
        lnc = np.where(Cf > 0, np.log(np.maximum(Cf, 1e-30)), -64.0)
        lnc = np.ascontiguousarray(lnc.astype(ml_dtypes.float8_e4m3))
        ident = np.eye(128, dtype=ml_dtypes.float8_e4m3)
        ct = np.ascontiguousarray(np.concatenate(
            [Cc[:, :CT_COLS].astype(np.uint8),
             st.view(np.uint8), ident.view(np.uint8)], axis=1))
        in_maps.append({"qe": qe, "embT": e2, "ct": ct, "lnc": lnc})
    return in_maps


def _run_device(in_maps, **kwargs):
    from concourse.bass_utils import run_bass_kernel_spmd

    if "nc" not in _CACHE:
        _CACHE["nc"] = _build_nc()
    return run_bass_kernel_spmd(_CACHE["nc"], in_maps,
                                core_ids=list(range(N_CORES)), **kwargs)


def _unshard(res):
    P = np.zeros((9, 1024), np.float64)
    for i in range(N_CORES):
        P += res.results[i]["o"].astype(np.float64)
    numer = np.zeros((OUT, BATCH), np.float64)
    denom = np.zeros(BATCH, np.float64)
    for bank in range(2):
        Pb = P[:, bank * 512:(bank + 1) * 512]
        for j in range(4):
            numer += Pb[2 * j:2 * j + 2, j * BATCH:(j + 1) * BATCH]
            denom += Pb[8, j * BATCH:(j + 1) * BATCH]
    out = (numer / denom[None, :]).T
    return np.ascontiguousarray(out, dtype=np.float32)


def kernel(q, k, embeddings, W, b, **_unused):
    in_maps = _prep_inputs(q, k, embeddings, W, b)
    res = _run_device(in_maps)
    return _unshard(res)
